# revision 1
# baseline (speedup 1.0000x reference)
"""MDTA Bass kernel for 8 TRN2 NeuronCores, two SPMD launches.

Reference semantics (row-major reshape!): q.reshape(B,HEADS,HW,D) maps
  tensor[b,hd,s,d] = conv[b, 16*hd+ci, y, 16*xs+d]  with s = ci*1024+y*8+xs
so the attention feature axis d is the LOW 4 BITS OF x, and
  attn[hd,d,j] = sum_{ci,y,xs} k2[16hd+ci,y,16xs+d]*q2[16hd+ci,y,16xs+j]
  out_conv[b,16hd+j, ci*8+y//16, (y%16)*8+xs] = sum_d v2[16hd+ci,y,16xs+d]*P[d,j]

Launch 1 (spatial shards: b x quarter-of-H, 1-row halo): LayerNorm, the three
conv1x1+conv3x3 chains (fp32r matmuls), and per-head 128x128 "pair" matrices
pair_hd[xk,xq] = sum_{ci,y,xs...} via two levels of PE transposes; outputs v2
slice + pairs. Host: extracts strip-diagonals -> gram, sums the 4 shards of
each batch, softmax -> P, builds block-diag PSTACK and per-output-row VROW
rearrangements of v (pure numpy data movement). Launch 2: one [128,128] matmul
per output row (PSTACK.T @ VROW = attention output row in conv layout), then
wo-conv1x1 + residual.
"""

import os
from contextlib import ExitStack

import numpy as np

import concourse.bacc as bacc
import concourse.bass as bass
import concourse.mybir as mybir
import concourse.tile as tile
from concourse import bass_utils

F32 = mybir.dt.float32
F32R = mybir.dt.float32r
AX = mybir.AxisListType
ALU = mybir.AluOpType
ACT = mybir.ActivationFunctionType

B, C, H, W = 2, 128, 128, 128
HEADS, D = 8, 16
EPS = 1e-5
RPC = H // 4
RH = RPC + 2
NPIX = RPC * W          # 4096
NHAL = RH * W           # 4352
WP = W + 2

_CACHE = {}


def _round_f32r(nc, pool, name, src_ap, shape, stage_pool=None):
    """DMA f32 DRAM -> staging SBUF, DVE-copy into an f32r-rounded tile."""
    sp = stage_pool if stage_pool is not None else pool
    stg = sp.tile(shape, F32, name=f"{name}_s", tag="wstage", bufs=2)
    nc.sync.dma_start(stg[:], src_ap[:])
    dst = pool.tile(shape, F32, name=name, tag=name)
    nc.vector.tensor_copy(dst.bitcast(F32R), stg[:])
    return dst


def _build_l1():
    nc = bacc.Bacc("TRN2", target_bir_lowering=False, debug=False, num_devices=8)
    x_d = nc.dram_tensor("x_sl", [128, NHAL], F32, kind="ExternalInput").ap()
    w1_d = {t: nc.dram_tensor(f"w{t}1T", [128, 128], F32, kind="ExternalInput").ap()
            for t in "qkv"}
    w2_d = {t: nc.dram_tensor(f"w{t}2T", [128, 9 * 128], F32, kind="ExternalInput").ap()
            for t in "qkv"}
    gm_d = nc.dram_tensor("gamma_b", [128, W], F32, kind="ExternalInput").ap()
    bt_d = nc.dram_tensor("beta_b", [128, W], F32, kind="ExternalInput").ap()
    idn_d = nc.dram_tensor("ident", [128, 128], F32, kind="ExternalInput").ap()
    v2_d = nc.dram_tensor("v2o", [128, NPIX], F32, kind="ExternalOutput").ap()
    pr_d = nc.dram_tensor("pairs", [128, 8 * 128], F32, kind="ExternalOutput").ap()

    with tile.TileContext(nc) as tc, ExitStack() as ctx:
        consts = ctx.enter_context(tc.tile_pool(name="consts", bufs=1))
        big = ctx.enter_context(tc.tile_pool(name="big", bufs=1))
        sbw = ctx.enter_context(tc.tile_pool(name="sbw", bufs=3))
        ps_cv = ctx.enter_context(tc.tile_pool(name="ps_cv", bufs=4, space="PSUM"))
        ps_tp = ctx.enter_context(tc.tile_pool(name="ps_tp", bufs=2, space="PSUM"))
        ps_pr = ctx.enter_context(tc.tile_pool(name="ps_pr", bufs=2, space="PSUM"))

        x_t = big.tile([128, NHAL], F32, name="x_t", tag="x_t")
        for a, b in ((0, 9), (9, 18), (18, 26), (26, 34)):
            nc.sync.dma_start(x_t[:, a * W:b * W], x_d[:, a * W:b * W])
        gm = consts.tile([128, W], F32, name="gm", tag="gm")
        nc.sync.dma_start(gm[:], gm_d[:])
        bt = consts.tile([128, W], F32, name="bt", tag="bt")
        nc.sync.dma_start(bt[:], bt_d[:])
        idn = consts.tile([128, 128], F32, name="idn", tag="idn")
        nc.sync.dma_start(idn[:], idn_d[:])
        w1 = {t: _round_f32r(nc, consts, f"w1{t}", w1_d[t], [128, 128]) for t in "qkv"}
        w2 = {t: _round_f32r(nc, consts, f"w2{t}", w2_d[t], [128, 9 * 128]) for t in "qkv"}

        # LayerNorm
        xvw = x_t.rearrange("p (r w) -> p r w", w=W)
        musum = sbw.tile([128, RH], F32, name="musum", tag="musum", bufs=1)
        nc.vector.reduce_sum(musum[:], xvw, axis=AX.X)
        ssq = sbw.tile([128, RH], F32, name="ssq", tag="ssq", bufs=1)
        for r in range(RH):
            scr = sbw.tile([128, W], F32, name=f"sqs{r}", tag="sqs", bufs=2)
            nc.scalar.activation(scr[:], x_t[:, r * W:(r + 1) * W], ACT.Square,
                                 accum_out=ssq[:, r:r + 1])
        mu = sbw.tile([128, RH], F32, name="mu", tag="mu", bufs=1)
        nc.vector.tensor_scalar_mul(mu[:], musum[:], 1.0 / W)
        var = sbw.tile([128, RH], F32, name="var", tag="var", bufs=1)
        nc.vector.tensor_scalar_mul(var[:], ssq[:], 1.0 / W)
        mu2 = sbw.tile([128, RH], F32, name="mu2", tag="mu2", bufs=1)
        nc.vector.tensor_tensor(mu2[:], mu[:], mu[:], op=ALU.mult)
        nc.vector.tensor_tensor(var[:], var[:], mu2[:], op=ALU.subtract)
        epst = sbw.tile([128, 1], F32, name="epst", tag="epst", bufs=1)
        nc.vector.memset(epst[:], EPS)
        std = sbw.tile([128, RH], F32, name="std", tag="std", bufs=1)
        nc.scalar.activation(std[:], var[:], ACT.Sqrt, bias=epst[:, 0:1])
        rstd = sbw.tile([128, RH], F32, name="rstd", tag="rstd", bufs=1)
        nc.vector.reciprocal(rstd[:], std[:])
        nmr = sbw.tile([128, RH], F32, name="nmr", tag="nmr", bufs=1)
        nc.vector.tensor_tensor(nmr[:], mu[:], rstd[:], op=ALU.mult)
        nc.vector.tensor_scalar_mul(nmr[:], nmr[:], -1.0)

        xn = big.tile([128, NHAL], F32, name="xn", tag="xn")
        for r in range(RH):
            seg = slice(r * W, (r + 1) * W)
            nc.scalar.activation(xn[:, seg].bitcast(F32R), x_t[:, seg], ACT.Identity,
                                 bias=nmr[:, r:r + 1], scale=rstd[:, r:r + 1])
            nc.vector.tensor_tensor(xn[:, seg].bitcast(F32R), xn[:, seg], gm[:],
                                    op=ALU.mult)
            nc.vector.tensor_tensor(xn[:, seg].bitcast(F32R), xn[:, seg], bt[:],
                                    op=ALU.add)

        conv_out = {}

        zrow = consts.tile([128, RH], F32, name="zrow", tag="zrow")
        nc.vector.memset(zrow[:], 0.0)

        def conv_chain(t):
            p1 = big.tile([128, RH * WP], F32, name=f"p1{t}", tag="p1", bufs=2)
            p1v = p1.rearrange("p (r w) -> p r w", w=WP)
            nc.vector.tensor_copy(p1v[:, :, 0:1].bitcast(F32R),
                                  zrow.rearrange("p (r o) -> p r o", o=1))
            nc.vector.tensor_copy(p1v[:, :, WP - 1:WP].bitcast(F32R),
                                  zrow.rearrange("p (r o) -> p r o", o=1))
            for g in range(9):
                rows = 4 if g < 8 else 2
                n = rows * W
                ps = ps_cv.tile([128, 512], F32, name=f"cv1{t}{g}", tag="cv")
                nc.tensor.matmul(ps[:, :n], w1[t].bitcast(F32R),
                                 xn[:, 4 * g * W:4 * g * W + n].bitcast(F32R),
                                 start=True, stop=True)
                nc.vector.tensor_copy(
                    p1v[:, 4 * g:4 * g + rows, 1:1 + W].bitcast(F32R),
                    ps[:, :n].rearrange("p (r w) -> p r w", w=W))
            dst = None
            if t != "v":
                dst = big.tile([128, NPIX], F32, name=f"c2{t}", tag=f"c2{t}")
            for g in range(8):
                ps2 = ps_cv.tile([128, 512], F32, name=f"cv3{t}{g}", tag="cv")
                for idx in range(9):
                    dy, dx = idx // 3, idx % 3
                    rhs = p1v[:, 4 * g + dy:4 * g + dy + 4, dx:dx + W]
                    nc.tensor.matmul(ps2[:],
                                     w2[t][:, idx * 128:(idx + 1) * 128].bitcast(F32R),
                                     rhs.bitcast(F32R),
                                     start=(idx == 0), stop=(idx == 8))
                if t == "v":
                    vch = sbw.tile([128, 512], F32, name=f"vch{g}", tag="vch")
                    nc.vector.tensor_copy(vch[:], ps2[:])
                    nc.sync.dma_start(v2_d[:, g * 512:(g + 1) * 512], vch[:])
                else:
                    nc.vector.tensor_copy(dst[:, g * 512:(g + 1) * 512], ps2[:])
            conv_out[t] = dst

        conv_chain("q")
        conv_chain("k")
        conv_chain("v")

        # level-1 transposes, written in (head, oct)-major layout:
        # tb[x, hd*512 + o*128 + y8*16 + ci] = c2[16hd+ci, (o*8+y8)*128 + x]
        tbig = {}
        for t in "qk":
            tb = big.tile([128, NPIX], F32, name=f"tb{t}", tag=f"tb{t}")
            tb5 = tb.rearrange("p (h o r c) -> p h o r c", h=8, o=4, r=8)
            for y in range(32):
                o, y8 = divmod(y, 8)
                tp = ps_tp.tile([128, 128], F32, name=f"tp{t}{y}", tag="tp")
                nc.tensor.transpose(tp[:], conv_out[t][:, y * 128:(y + 1) * 128], idn[:])
                nc.vector.tensor_copy(tb5[:, :, o, y8, :],
                                      tp.rearrange("p (h c) -> p h c", c=16))
            tbig[t] = tb

        # per head: level-2 transposes + pair matmuls
        pair_sb = big.tile([128, 8 * 128], F32, name="pair_sb", tag="pair_sb")
        for hd in range(8):
            pps = ps_pr.tile([128, 128], F32, name=f"pps{hd}", tag="pps")
            for o in range(4):
                koq = {}
                for t in "qk":
                    src = tbig[t].rearrange("p (h o f) -> p h o f", h=8, o=4)
                    chunk = src[:, hd, o, :]
                    tp2 = ps_tp.tile([128, 128], F32, name=f"t2{t}{hd}{o}", tag="tp")
                    nc.tensor.transpose(tp2[:], chunk, idn[:])
                    sb2 = sbw.tile([128, 128], F32, name=f"s2{t}{hd}{o}", tag=f"s2{t}")
                    nc.vector.tensor_copy(sb2[:], tp2[:])
                    koq[t] = sb2
                nc.tensor.matmul(pps[:], koq["k"][:], koq["q"][:],
                                 start=(o == 0), stop=(o == 3))
            nc.vector.tensor_copy(pair_sb[:, hd * 128:(hd + 1) * 128], pps[:])
        for hh in range(4):
            nc.sync.dma_start(pr_d[:, hh * 256:(hh + 1) * 256],
                              pair_sb[:, hh * 256:(hh + 1) * 256])

    nc.compile()
    return nc


def _build_l2():
    nc = bacc.Bacc("TRN2", target_bir_lowering=False, debug=False, num_devices=8)
    pst_d = nc.dram_tensor("pstack", [128, 128], F32, kind="ExternalInput").ap()
    vr_d = nc.dram_tensor("vrows", [128, NPIX], F32, kind="ExternalInput").ap()
    xr_d = nc.dram_tensor("x_res", [128, NPIX], F32, kind="ExternalInput").ap()
    wo_d = nc.dram_tensor("woT", [128, 128], F32, kind="ExternalInput").ap()
    y_d = nc.dram_tensor("y_sl", [128, NPIX], F32, kind="ExternalOutput").ap()

    with tile.TileContext(nc) as tc, ExitStack() as ctx:
        consts = ctx.enter_context(tc.tile_pool(name="consts", bufs=1))
        big = ctx.enter_context(tc.tile_pool(name="big", bufs=1))
        sbw = ctx.enter_context(tc.tile_pool(name="sbw", bufs=3))
        ps_a = ctx.enter_context(tc.tile_pool(name="ps_a", bufs=4, space="PSUM"))

        pst = consts.tile([128, 128], F32, name="pst", tag="pst")
        nc.sync.dma_start(pst[:], pst_d[:])
        wo = _round_f32r(nc, consts, "wo", wo_d, [128, 128])
        vr = big.tile([128, NPIX], F32, name="vr", tag="vr")
        xr = big.tile([128, NPIX], F32, name="xr", tag="xr")
        for g in range(4):
            seg = slice(g * 1024, (g + 1) * 1024)
            nc.sync.dma_start(vr[:, seg], vr_d[:, seg])
            nc.sync.dma_start(xr[:, seg], xr_d[:, seg])

        oat = big.tile([128, NPIX], F32, name="oat", tag="oat")
        for y in range(32):
            seg = slice(y * 128, (y + 1) * 128)
            ps = ps_a.tile([128, 128], F32, name=f"ar{y}", tag="ar")
            nc.tensor.matmul(ps[:], pst[:], vr[:, seg], start=True, stop=True)
            nc.vector.tensor_copy(oat[:, seg].bitcast(F32R), ps[:])
        for g in range(8):
            seg = slice(g * 512, (g + 1) * 512)
            ps4 = ps_a.tile([128, 512], F32, name=f"fin{g}", tag="fin")
            nc.tensor.matmul(ps4[:], wo.bitcast(F32R), oat[:, seg].bitcast(F32R),
                             start=True, stop=True)
            ysb = sbw.tile([128, 512], F32, name=f"ysb{g}", tag="ysb")
            nc.vector.tensor_tensor(ysb[:], ps4[:], xr[:, seg], op=ALU.add)
            nc.sync.dma_start(y_d[:, seg], ysb[:])

    nc.compile()
    return nc


def _get(name):
    if name not in _CACHE:
        _CACHE[name] = _build_l1() if name == "l1" else _build_l2()
    return _CACHE[name]


def _host_middle(pairs_list, v2o_list, scale):
    """pairs -> softmax P + PSTACK; v2o -> full v_conv -> per-core VROWS."""
    f = np.float32
    G = np.zeros((B, HEADS, D, D), f)
    for c in range(8):
        pr = pairs_list[c].reshape(128, 8, 128)
        for hd in range(HEADS):
            blk = pr[:, hd, :].reshape(8, 16, 8, 16)      # [xs, d, xs', j]
            G[c // 4, hd] += np.einsum("adaj->dj", blk)
    G /= float(np.asarray(scale, f)[0])
    Gm = G - G.max(-1, keepdims=True)
    E = np.exp(Gm)
    P = (E / E.sum(-1, keepdims=True)).astype(f)          # [B, HEADS, 16, 16]

    pstack = np.zeros((B, 128, 128), f)
    for b in range(B):
        for hd in range(HEADS):
            pstack[b, 16 * hd:16 * hd + 16, 16 * hd:16 * hd + 16] = P[b, hd]

    v_conv = np.empty((B, C, H, W), f)
    for c in range(8):
        b, r0 = c // 4, 32 * (c % 4)
        v_conv[b, :, r0:r0 + RPC, :] = v2o_list[c].reshape(C, RPC, W)
    # vc[b, hd, ci, y, xs, d]
    vc = v_conv.reshape(B, HEADS, 16, H, 8, 16)
    vrows = []
    for c in range(8):
        b, r0 = c // 4, 32 * (c % 4)
        rows = np.empty((32, 128, 128), f)
        for i in range(32):
            yp = r0 + i
            ci, yb = yp // 8, yp % 8
            blk = vc[b, :, ci, 16 * yb:16 * yb + 16, :, :]   # [hd, yy, xs, d]
            rows[i] = blk.transpose(0, 3, 1, 2).reshape(128, 128)
        vrows.append(np.ascontiguousarray(rows.transpose(1, 0, 2))
                     .reshape(128, NPIX))
    return pstack, vrows


def _maps_l1(x, gamma, beta, wq1, wq2, wk1, wk2, wv1, wv2):
    f = np.float32
    xp = np.pad(np.asarray(x, f), ((0, 0), (0, 0), (1, 1), (0, 0)))
    common = {
        "gamma_b": np.broadcast_to(np.asarray(gamma, f), (128, W)).copy(),
        "beta_b": np.broadcast_to(np.asarray(beta, f), (128, W)).copy(),
        "ident": np.eye(128, dtype=f),
    }
    for t, w1_, w2_ in (("q", wq1, wq2), ("k", wk1, wk2), ("v", wv1, wv2)):
        common[f"w{t}1T"] = np.ascontiguousarray(np.asarray(w1_, f)[:, :, 0, 0].T)
        common[f"w{t}2T"] = np.ascontiguousarray(
            np.asarray(w2_, f).transpose(1, 2, 3, 0).reshape(128, 9 * 128))
    maps = []
    for c in range(8):
        b, r0 = c // 4, 32 * (c % 4)
        m = dict(common)
        m["x_sl"] = np.ascontiguousarray(xp[b, :, r0:r0 + RH, :].reshape(128, NHAL))
        maps.append(m)
    return maps


def _run(nc, maps, key):
    trace = bool(int(os.environ.get("KERNEL_TRACE", "0")))
    if _CACHE.get("sim"):
        from concourse.bass_interp import MultiCoreSim
        sim = MultiCoreSim(nc, num_cores=8, require_finite=True, require_nnan=True)
        cores = list(sim.cores.values())
        for c, m in enumerate(maps):
            for k, v in m.items():
                cores[c].tensor(k)[:] = v
        sim.simulate(check_with_hw=False)
        return [{k: np.array(cores[c].tensor(k)) for k in key} for c in range(8)]
    res = bass_utils.run_bass_kernel_spmd(nc, maps, core_ids=list(range(8)),
                                          trace=trace)
    _CACHE.setdefault("results", []).append(res)
    return res.results


def kernel(x, gamma, beta, scale, wq1, wq2, wk1, wk2, wv1, wv2, wo):
    f = np.float32
    r1 = _run(_get("l1"), _maps_l1(x, gamma, beta, wq1, wq2, wk1, wk2, wv1, wv2),
              ("v2o", "pairs"))
    pstack, vrows = _host_middle([r["pairs"] for r in r1],
                                 [r["v2o"] for r in r1], scale)
    woT = np.ascontiguousarray(np.asarray(wo, f)[:, :, 0, 0].T)
    xf = np.asarray(x, f)
    maps2 = []
    for c in range(8):
        b, r0 = c // 4, 32 * (c % 4)
        maps2.append({
            "pstack": pstack[b],
            "vrows": vrows[c],
            "x_res": np.ascontiguousarray(xf[b, :, r0:r0 + RPC, :].reshape(128, NPIX)),
            "woT": woT,
        })
    r2 = _run(_get("l2"), maps2, ("y_sl",))
    y = np.empty((B, C, H, W), f)
    for c in range(8):
        b, r0 = c // 4, 32 * (c % 4)
        y[b, :, r0:r0 + RPC, :] = r2[c]["y_sl"].reshape(C, RPC, W)
    return y


def kernel_sim(**inputs):
    _CACHE["sim"] = True
    try:
        return kernel(**inputs)
    finally:
        _CACHE["sim"] = False

